# revision 1
# baseline (speedup 1.0000x reference)
"""Trainium2 Bass kernel for the CCG supertagger BERT model.

Data-parallel over batch: 16 samples -> 8 cores x 2 samples.
Activations kept transposed [H (6 chunks of 128), T=512 tokens] in SBUF.
fp32r matmuls for the residual-stream projections; bf16 for attention
internals / Wo2 / head-w2 (fp32->bf16 cast done inside gpsimd DMA).
"""
import numpy as np

import concourse.bass as bass
import concourse.tile as tile
from concourse import bacc, mybir
from concourse.bass_utils import run_bass_kernel_spmd
from concourse.masks import make_identity

F32 = mybir.dt.float32
F32R = mybir.dt.float32r
BF16 = mybir.dt.bfloat16
I32 = mybir.dt.int32
AF = mybir.ActivationFunctionType
ALU = mybir.AluOpType

B, S, W = 16, 256, 128
V, H, L, NH, DH, FF, C = 30522, 768, 12, 12, 64, 3072, 425
EPS = 1e-12
N_CORES = 8
BPC = B // N_CORES          # samples per core
T = BPC * S                 # tokens per core (512)
HC = H // 128               # 6
FFC = FF // 128             # 24
TC = T // 128               # 4 token chunks
M1 = 1024
M1C = M1 // 128             # 8
CPAD = 448                  # padded class dim for sbuf tiles


DEBUG_TAPS = False


def build_program(n_layers=L):
    nc = bacc.Bacc("TRN2", target_bir_lowering=False, debug=False,
                   num_devices=N_CORES)

    dt_ = lambda name, shape, dt, kind: nc.dram_tensor(name, shape, dt, kind=kind).ap()
    # per-core sharded inputs
    enc = dt_("enc", [T, 1], I32, "ExternalInput")
    ab = dt_("ab", [T], F32, "ExternalInput")            # attn bias (per key pos)
    pmat = dt_("pmat", [T, 128], F32, "ExternalInput")   # pooling matrices
    # replicated model inputs
    word_emb = dt_("word_emb", [V, H], F32, "ExternalInput")
    pos_emb = dt_("pos_emb", [S, H], F32, "ExternalInput")
    type_emb = dt_("type_emb", [1, H], F32, "ExternalInput")
    emb_ln_s = dt_("emb_ln_s", [H], F32, "ExternalInput")
    emb_ln_b = dt_("emb_ln_b", [H], F32, "ExternalInput")
    Wq = dt_("Wq", [L, H, H], F32, "ExternalInput")
    bq = dt_("bq", [L, H], F32, "ExternalInput")
    Wk = dt_("Wk", [L, H, H], F32, "ExternalInput")
    bk = dt_("bk", [L, H], F32, "ExternalInput")
    Wv = dt_("Wv", [L, H, H], F32, "ExternalInput")
    bv = dt_("bv", [L, H], F32, "ExternalInput")
    Wo = dt_("Wo", [L, H, H], F32, "ExternalInput")
    bo = dt_("bo", [L, H], F32, "ExternalInput")
    ln1_s = dt_("ln1_s", [L, H], F32, "ExternalInput")
    ln1_b = dt_("ln1_b", [L, H], F32, "ExternalInput")
    Wi = dt_("Wi", [L, H, FF], F32, "ExternalInput")
    bi = dt_("bi", [L, FF], F32, "ExternalInput")
    Wo2 = dt_("Wo2", [L, FF, H], F32, "ExternalInput")
    bo2 = dt_("bo2", [L, H], F32, "ExternalInput")
    ln2_s = dt_("ln2_s", [L, H], F32, "ExternalInput")
    ln2_b = dt_("ln2_b", [L, H], F32, "ExternalInput")
    w1 = dt_("w1", [H, M1], F32, "ExternalInput")
    b1 = dt_("b1", [M1], F32, "ExternalInput")
    w2 = dt_("w2", [M1, C], F32, "ExternalInput")
    b2 = dt_("b2", [C], F32, "ExternalInput")
    cones = dt_("cones", [128, 1], F32, "ExternalInput")   # column of ones
    crow = dt_("crow", [1, 128], F32, "ExternalInput")     # row of ones
    ceps = dt_("ceps", [128, 1], F32, "ExternalInput")     # EPS constant
    out_d = dt_("out", [T, C], F32, "ExternalOutput")
    if DEBUG_TAPS:
        dbg_x0 = dt_("dbg_x0", [H, T], F32, "ExternalOutput")
        dbg_q = dt_("dbg_q", [H, T], BF16, "ExternalOutput")
        dbg_ctx = dt_("dbg_ctx", [T, H], BF16, "ExternalOutput")
        dbg_a = dt_("dbg_a", [H, T], F32, "ExternalOutput")
        dbg_h2 = dt_("dbg_h2", [H, T], F32, "ExternalOutput")
        dbg_f2 = dt_("dbg_f2", [H, T], F32, "ExternalOutput")
        dbg_x1 = dt_("dbg_x1", [H, T], F32, "ExternalOutput")
        dbg_exp0 = dt_("dbg_exp0", [128, 256], BF16, "ExternalOutput")
        dbg_exp1 = dt_("dbg_exp1", [128, 256], BF16, "ExternalOutput")
        dbg_rec0 = dt_("dbg_rec0", [128, 1], F32, "ExternalOutput")
        dbg_v = dt_("dbg_v", [T, H], BF16, "ExternalOutput")

    with tile.TileContext(nc) as tc:
        _emit(nc, tc, n_layers, locals())
    nc.compile()
    return nc


def _emit(nc, tc, n_layers, d):
    from contextlib import ExitStack
    ctx = ExitStack()
    with ctx:
        _emit_body(nc, tc, n_layers, d, ctx)


def _emit_body(nc, tc, n_layers, d, ctx):
    pool = lambda name, bufs, space="SBUF": ctx.enter_context(
        tc.tile_pool(name=name, bufs=bufs, space=space))

    p_xt = pool("xt", 3)          # [128, HC, 512] f32r residual-stream acts
    p_xb = pool("xb", 2)          # [128, HC, 512] bf16 rhs copies (Xb, H2b)
    p_qk = pool("qk", 3)          # [128, HC, 512] bf16 (QT, KT, VT/ctxT, reluT)
    p_v = pool("v", 1)            # [128, TC, 768] bf16 token-major V
    p_ctx = pool("ctxp", 1)       # [128, TC, 768] bf16 token-major ctx
    p_exp = pool("exp", 4)        # [128, 256] bf16 exp tiles
    p_scr = pool("scr", 3)        # [128, 768] f32 scratch
    p_gel = pool("gel", 1)        # [128, 6, 512] bf16 gelu quarter
    p_f2a = pool("f2a", 1)        # [128, HC, 512] f32 FFN accum / emb x0 / w1 / f2sb
    p_w6 = pool("w6", 2)          # [128, HC, 768] bf16 whole QKVO weights
    p_wi = pool("wi", 2)          # [128, HC, 768] bf16 Wi quarters / head w2
    p_wb = pool("wb", 8)          # [128, 768] bf16 Wo2 k-bands
    p_f1 = pool("f1", 1)          # [128, M1C, 512] bf16 head f1relu
    p_bias = pool("bias", 6)      # [128, 24] f32 per-partition bias/scale tiles
    p_vec = pool("vec", 3)        # [1, 512] f32 LN stat vectors
    p_vec2 = pool("vec2", 1)      # [1, 1024] f32 (rstd | -mu*rstd)
    p_lnbc = pool("lnbc", 2)      # [128, 1024] f32 broadcast LN stats / b2bc
    p_dram = pool("dram", 2, "DRAM")
    p_sm = pool("sm", 2)          # small per-chunk scalars
    p_cst = pool("cst", 1)        # constants
    p_pos = pool("pos", 1)

    ps_mm = pool("ps_mm", 3, "PSUM")    # [128, 512]
    ps_sc = pool("ps_sc", 2, "PSUM")    # [128, 256] scores / [1, 512] LN stats
    ps_cx = pool("ps_cx", 2, "PSUM")    # [128, 64] ctx
    ps_su = pool("ps_su", 1, "PSUM")    # [128, 1] softmax sums

    enc, ab, pmat = d["enc"], d["ab"], d["pmat"]
    word_emb, pos_emb, type_emb = d["word_emb"], d["pos_emb"], d["type_emb"]
    emb_ln_s, emb_ln_b = d["emb_ln_s"], d["emb_ln_b"]
    out_d = d["out_d"]

    # ---- constants ----
    ident = p_cst.tile([128, 128], BF16, tag="ident")
    make_identity(nc, ident[:])
    identf = p_cst.tile([128, 128], F32, tag="identf")
    make_identity(nc, identf[:])
    ones_c = p_cst.tile([128, 1], F32R, tag="ones_c")
    nc.sync.dma_start(ones_c[:], d["cones"][:].bitcast(F32R))
    ones_cb = p_cst.tile([128, 1], BF16, tag="ones_cb")
    nc.gpsimd.dma_start(ones_cb[:], d["cones"][:])
    ones_r = p_cst.tile([1, 128], F32R, tag="ones_r")
    nc.sync.dma_start(ones_r[:], d["crow"][:].bitcast(F32R))
    eps_t = p_cst.tile([128, 1], F32, tag="eps")
    nc.sync.dma_start(eps_t[:], d["ceps"][:])

    # attn bias as [128, TC]
    ab_t = p_cst.tile([128, TC], F32, tag="ab")
    nc.sync.dma_start(ab_t[:], ab.rearrange("(c p) -> p c", p=128))

    def ln_pair(ap_s, ap_b, tag):
        t = p_bias.tile([128, 2 * HC], F32, tag="bias")
        nc.sync.dma_start(t[:, 0:HC], ap_s.rearrange("(c p) -> p c", p=128))
        nc.sync.dma_start(t[:, HC:2 * HC], ap_b.rearrange("(c p) -> p c", p=128))
        return t

    # =============== embedding ===============
    x0 = p_f2a.tile([128, TC, H], F32, tag="f2a")
    for c in range(TC):
        idx_t = p_sm.tile([128, 1], I32, tag="idx")
        nc.sync.dma_start(idx_t[:], enc[128 * c:128 * (c + 1), :])
        nc.gpsimd.indirect_dma_start(
            out=x0[:, c, :], out_offset=None, in_=word_emb[:],
            in_offset=bass.IndirectOffsetOnAxis(ap=idx_t[:, :1], axis=0))
    pos_t = p_pos.tile([128, 2, H], F32, tag="pos")
    nc.sync.dma_start(pos_t[:, 0, :], pos_emb[0:128, :])
    nc.sync.dma_start(pos_t[:, 1, :], pos_emb[128:256, :])
    typ_t = p_pos.tile([128, H], F32, tag="typ")
    nc.sync.dma_start(typ_t[:], type_emb[0:1, :].partition_broadcast(128)[:, 0, :])

    emb_sb = ln_pair(emb_ln_s, emb_ln_b, "embln")
    X = p_xt.tile([128, HC, T], F32R, tag="xt")
    for c in range(TC):
        xc = x0[:, c, :]
        nc.vector.tensor_tensor(out=xc, in0=xc, in1=pos_t[:, c % 2, :], op=ALU.add)
        nc.vector.tensor_tensor(out=xc, in0=xc, in1=typ_t[:], op=ALU.add)
        # layernorm over free dim (token-major)
        su = p_sm.tile([128, 4], F32, tag="stat")
        nc.vector.reduce_sum(out=su[:, 0:1], in_=xc, axis=mybir.AxisListType.X)
        sq = p_scr.tile([128, H], F32, tag="scr")
        nc.scalar.activation(sq[:], xc, AF.Square, accum_out=su[:, 1:2])
        st = p_sm.tile([128, 4], F32, tag="stat2")
        nc.vector.tensor_scalar_mul(st[:, 0:1], su[:, 0:1], 1.0 / H)      # mu
        nc.vector.tensor_scalar_mul(st[:, 1:2], su[:, 1:2], 1.0 / H)      # m2
        nc.vector.tensor_tensor(out=st[:, 2:3], in0=st[:, 0:1], in1=st[:, 0:1], op=ALU.mult)
        nc.vector.tensor_tensor(out=st[:, 3:4], in0=st[:, 1:2], in1=st[:, 2:3], op=ALU.subtract)
        sd = p_sm.tile([128, 2], F32, tag="stat3")
        nc.scalar.activation(sd[:, 0:1], st[:, 3:4], AF.Sqrt, bias=eps_t[:, 0:1])
        nc.vector.reciprocal(sd[:, 1:2], sd[:, 0:1])
        # x_hat = (x - mu) * rstd
        nc.vector.tensor_scalar(out=xc, in0=xc, scalar1=st[:, 0:1], scalar2=sd[:, 1:2],
                                op0=ALU.subtract, op1=ALU.mult)
        # transpose into X_T (fp32), fused scale/bias
        for k in range(HC):
            pt = ps_mm.tile([128, 128], F32, tag="ps_mm")
            nc.tensor.transpose(pt[:], xc[:, 128 * k:128 * (k + 1)], identf[:])
            nc.scalar.activation(X[:, k, 128 * c:128 * (c + 1)], pt[:],
                                 AF.Identity, scale=emb_sb[:, k:k + 1],
                                 bias=emb_sb[:, HC + k:HC + k + 1])

    def tap(name, tile_, nchunks, dt=F32):
        if not DEBUG_TAPS or name not in d:
            return
        ap = d[name]
        for k in range(nchunks):
            nc.sync.dma_start(ap[128 * k:128 * (k + 1), :],
                              tile_[:, k, :] if dt is None else tile_[:, k, :].bitcast(dt))

    tap("dbg_x0", X, HC)
    # =============== transformer layers ===============
    for l in range(n_layers):
        X = _layer(nc, tc, d, l, X, dict(
            p_xt=p_xt, p_qk=p_qk, p_v=p_v, p_ctx=p_ctx, p_exp=p_exp,
            p_scr=p_scr, p_gel=p_gel, p_f2a=p_f2a, p_w6=p_w6, p_wi=p_wi, p_wb=p_wb, p_xb=p_xb,
            p_bias=p_bias, p_vec=p_vec, p_vec2=p_vec2, p_lnbc=p_lnbc, p_dram=p_dram, p_sm=p_sm,
            ps_mm=ps_mm, ps_sc=ps_sc, ps_cx=ps_cx, ps_su=ps_su,
            ones_c=ones_c, ones_cb=ones_cb, ones_r=ones_r, ab_t=ab_t, eps_t=eps_t, tap=tap, dd=d,
            ident=ident, ln_pair=ln_pair))

    if DEBUG_TAPS:
        tap("dbg_x1", X, HC)
    # =============== head ===============
    _head(nc, tc, d, X, dict(
        p_qk=p_qk, p_f1=p_f1, p_f2a=p_f2a, p_wi=p_wi, p_lnbc=p_lnbc,
        p_scr=p_scr, p_bias=p_bias, p_sm=p_sm,
        ps_mm=ps_mm, pmat=pmat, out_d=out_d))


def _wfull(nc, pool, tag, src2d, ncols, col0=0, nk=HC):
    """Whole k-major weight tile [128, nk, ncols] (bf16, cast in DMA):
    (p, k, f) <- src2d[128k+p, col0+f] -- contiguous ncols*4B runs."""
    t = pool.tile([128, nk, ncols], BF16, tag=tag)
    src = src2d.rearrange("(k p) f -> p k f", p=128)[:, :, col0:col0 + ncols]
    nc.gpsimd.dma_start(t[:], src)
    return t


def _layer(nc, tc, d, l, X, e):
    p_xt, p_qk, p_v, p_ctx, p_exp = e["p_xt"], e["p_qk"], e["p_v"], e["p_ctx"], e["p_exp"]
    p_scr, p_gel, p_f2a, p_wb = e["p_scr"], e["p_gel"], e["p_f2a"], e["p_wb"]
    p_w6, p_wi, p_xb = e["p_w6"], e["p_wi"], e["p_xb"]
    p_bias, p_vec, p_sm = e["p_bias"], e["p_vec"], e["p_sm"]
    ps_mm, ps_sc, ps_cx, ps_su = e["ps_mm"], e["ps_sc"], e["ps_cx"], e["ps_su"]
    ones_c, ones_cb, ones_r, ab_t = e["ones_c"], e["ones_cb"], e["ones_r"], e["ab_t"]
    dd = e["dd"]
    ident, ln_pair = e["ident"], e["ln_pair"]

    qkvo_b = p_bias.tile([128, 4 * HC], F32, tag="bias")
    for i, bap in enumerate([d["bq"], d["bk"], d["bv"], d["bo"]]):
        nc.sync.dma_start(qkvo_b[:, i * HC:(i + 1) * HC],
                          bap[l].rearrange("(c p) -> p c", p=128))

    # bf16 copy of X for use as the moving operand
    Xb = p_xb.tile([128, HC, T], BF16, tag="xb")
    for k in range(HC):
        nc.vector.tensor_copy(Xb[:, k, :], X[:, k, :].bitcast(F32))

    # ---- Q, K, V projections (transposed layout), V then transposed to token-major
    QT = p_qk.tile([128, HC, T], BF16, tag="qk")
    KT = p_qk.tile([128, HC, T], BF16, tag="qk")
    VT = p_qk.tile([128, HC, T], BF16, tag="qk")
    for w_ap, dst, boff in [(d["Wq"], QT, 0), (d["Wk"], KT, HC),
                            (d["Wv"], VT, 2 * HC)]:
        wt = _wfull(nc, p_w6, "w6", w_ap[l], H)
        for m in range(HC):
            pm_ = ps_mm.tile([128, T], F32, tag="ps_mm")
            for k in range(HC):
                nc.tensor.matmul(pm_[:], wt[:, k, 128 * m:128 * (m + 1)], Xb[:, k, :],
                                 start=(k == 0), stop=(k == HC - 1))
            nc.scalar.activation(dst[:, m, :], pm_[:], AF.Identity,
                                 bias=qkvo_b[:, boff + m:boff + m + 1])

    if l == 0:
        e["tap"]("dbg_q", QT, HC, BF16)
    # V -> token-major [128, TC, H] bf16 via PE transpose
    Vtok = p_v.tile([128, TC, H], BF16, tag="v")
    for c in range(TC):
        for k in range(HC):
            pt = ps_mm.tile([128, 128], BF16, tag="ps_mm")
            nc.tensor.transpose(pt[:], VT[:, k, 128 * c:128 * (c + 1)], ident[:])
            nc.scalar.activation(Vtok[:, c, 128 * k:128 * (k + 1)], pt[:], AF.Copy)

    if l == 0:
        e["tap"]("dbg_v", Vtok, TC, BF16)
    # ---- attention ----
    # ctx accumulated token-major with a fused softmax-sum column, then
    # transposed back to [H, T] layout for the O-projection.
    ctok = p_ctx.tile([128, TC, H], BF16, tag="ctxp")
    for s in range(BPC):
        for h in range(NH):
            kc, po = h // 2, 64 * (h % 2)
            exp_t = [None, None]
            for j in range(2):
                psc = ps_sc.tile([128, 256], F32, tag="ps_sc")
                nc.tensor.matmul(
                    psc[:],
                    KT[po:po + 64, kc, 256 * s + 128 * j:256 * s + 128 * (j + 1)],
                    QT[po:po + 64, kc, 256 * s:256 * (s + 1)],
                    start=True, stop=True)
                et = p_exp.tile([128, 256], BF16, tag="exp")
                nc.scalar.activation(et[:], psc[:], AF.Exp, scale=0.125,
                                     bias=ab_t[:, 2 * s + j:2 * s + j + 1])
                exp_t[j] = et
            if l == 0 and s == 0 and h == 0 and "dbg_exp0" in dd:
                nc.sync.dma_start(dd["dbg_exp0"][:], exp_t[0][:])
                nc.sync.dma_start(dd["dbg_exp1"][:], exp_t[1][:])
            for i in range(2):            # query chunks
                pcx = ps_cx.tile([128, 64], F32, tag="ps_cx")
                psu = ps_su.tile([128, 1], F32, tag="ps_su")
                for j in range(2):
                    lhs = exp_t[j][:, 128 * i:128 * (i + 1)]
                    nc.tensor.matmul(pcx[:], lhs,
                                     Vtok[:, 2 * s + j, 64 * h:64 * h + 64],
                                     start=(j == 0), stop=(j == 1))
                    nc.tensor.matmul(psu[:], lhs, ones_cb[:],
                                     start=(j == 0), stop=(j == 1))
                rec = p_sm.tile([128, 1], F32, tag="rec")
                nc.vector.reciprocal(rec[:], psu[:])
                if l == 0 and s == 0 and h == 0 and i == 0 and "dbg_rec0" in dd:
                    nc.sync.dma_start(dd["dbg_rec0"][:], rec[:])
                nc.vector.tensor_scalar_mul(
                    ctok[:, 2 * s + i, 64 * h:64 * h + 64], pcx[:], rec[:])
    if l == 0:
        e["tap"]("dbg_ctx", ctok, TC, BF16)
    # transpose ctx back to [H, T]
    ctxT = p_qk.tile([128, HC, T], BF16, tag="qk")
    for c in range(TC):
        for k in range(HC):
            pt = ps_mm.tile([128, 128], BF16, tag="ps_mm")
            nc.tensor.transpose(pt[:], ctok[:, c, 128 * k:128 * (k + 1)], ident[:])
            nc.scalar.activation(ctxT[:, k, 128 * c:128 * (c + 1)], pt[:], AF.Copy)

    # ---- O-projection + residual + LN1 ----
    ln1 = ln_pair(d["ln1_s"][l], d["ln1_b"][l], "ln1")
    A = p_xt.tile([128, HC, T], F32R, tag="xt")
    wo_t = _wfull(nc, p_w6, "w6", d["Wo"][l], H)
    for m in range(HC):
        pm_ = ps_mm.tile([128, T], F32, tag="ps_mm")
        for k in range(HC):
            nc.tensor.matmul(pm_[:], wo_t[:, k, 128 * m:128 * (m + 1)], ctxT[:, k, :],
                             start=(k == 0), stop=(k == HC - 1))
        t1 = p_scr.tile([128, T], F32, tag="scr")
        nc.scalar.activation(t1[:], pm_[:], AF.Identity,
                             bias=qkvo_b[:, 3 * HC + m:3 * HC + m + 1])
        nc.vector.tensor_tensor(out=A[:, m, :], in0=t1[:], in1=X[:, m, :].bitcast(F32),
                                op=ALU.add)
    if l == 0:
        e["tap"]("dbg_a", A, HC)
    H2 = _ln_t(nc, A, ln1, e)
    if l == 0:
        e["tap"]("dbg_h2", H2, HC)

    # ---- FFN (quarter passes over FF) ----
    ln2 = ln_pair(d["ln2_s"][l], d["ln2_b"][l], "ln2")
    bi_t = p_bias.tile([128, FFC], F32, tag="bias")
    nc.sync.dma_start(bi_t[:], d["bi"][l].rearrange("(c p) -> p c", p=128))
    bo2_t = p_bias.tile([128, HC], F32, tag="bias")
    nc.sync.dma_start(bo2_t[:], d["bo2"][l].rearrange("(c p) -> p c", p=128))

    H2b = p_xb.tile([128, HC, T], BF16, tag="xb")
    for k in range(HC):
        nc.vector.tensor_copy(H2b[:, k, :], H2[:, k, :].bitcast(F32))

    F2 = p_f2a.tile([128, HC, T], F32, tag="f2a")
    NQ = 4
    QK = FFC // NQ                      # 6 ff-chunks per quarter
    for q in range(NQ):
        wi_t = _wfull(nc, p_wi, "wi", d["Wi"][l], 128 * QK, col0=128 * QK * q)
        gel = p_gel.tile([128, QK, T], BF16, tag="gel")
        for mq in range(QK):
            m = q * QK + mq
            pm_ = ps_mm.tile([128, T], F32, tag="ps_mm")
            for k in range(HC):
                nc.tensor.matmul(pm_[:], wi_t[:, k, 128 * mq:128 * (mq + 1)],
                                 H2b[:, k, :], start=(k == 0), stop=(k == HC - 1))
            nc.scalar.activation(gel[:, mq, :], pm_[:], AF.Gelu,
                                 bias=bi_t[:, m:m + 1])
        wbs = []
        for kq in range(QK):
            m = q * QK + kq
            wb = p_wb.tile([128, 768], BF16, tag="wb")
            nc.gpsimd.dma_start(wb[:], d["Wo2"][l, 128 * m:128 * (m + 1), :])
            wbs.append(wb)
        for o in range(HC):
            pm_ = ps_mm.tile([128, T], F32, tag="ps_mm")
            for kq in range(QK):
                nc.tensor.matmul(pm_[:], wbs[kq][:, 128 * o:128 * (o + 1)],
                                 gel[:, kq, :],
                                 start=(kq == 0), stop=(kq == QK - 1))
            if q == 0:
                nc.scalar.activation(F2[:, o, :], pm_[:], AF.Copy)
            else:
                nc.vector.tensor_tensor(out=F2[:, o, :], in0=F2[:, o, :], in1=pm_[:],
                                        op=ALU.add)
    # residual + bias
    Apre = p_xt.tile([128, HC, T], F32R, tag="xt")
    for o in range(HC):
        t1 = p_scr.tile([128, T], F32, tag="scr")
        nc.vector.tensor_scalar_add(t1[:], F2[:, o, :], bo2_t[:, o:o + 1])
        nc.vector.tensor_tensor(out=Apre[:, o, :], in0=t1[:],
                                in1=H2[:, o, :].bitcast(F32), op=ALU.add)
    if l == 0:
        e["tap"]("dbg_f2", Apre, HC)
    return _ln_t(nc, Apre, ln2, e)


def _ln_t(nc, A, ln_sb, e):
    """LayerNorm over the partition (H) dim for transposed activations.
    A: [128, HC, T] f32r tile. ln_sb: [128, 2*HC] (scale | bias).
    Returns new [128, HC, T] f32r tile."""
    p_xt, p_scr = e["p_xt"], e["p_scr"]
    ps_mm = e["ps_mm"]
    ones_c, ones_r = e["ones_c"], e["ones_r"]

    pmean = e["ps_sc"].tile([1, T], F32, tag="ps_sc")
    for k in range(HC):
        nc.tensor.matmul(pmean[:], ones_c[:], A[:, k, :],
                         start=(k == 0), stop=(k == HC - 1))
    psq = e["ps_sc"].tile([1, T], F32, tag="ps_sc")
    for k in range(HC):
        sq = p_scr.tile([128, T], F32R, tag="scr")
        nc.scalar.activation(sq[:], A[:, k, :].bitcast(F32), AF.Square)
        nc.tensor.matmul(psq[:], ones_c[:], sq[:],
                         start=(k == 0), stop=(k == HC - 1))
    va = e["p_vec"].tile([1, T], F32, tag="vec")   # mu
    vb = e["p_vec"].tile([1, T], F32, tag="vec")   # m2 -> var
    vc = e["p_vec"].tile([1, T], F32, tag="vec")   # musq -> sd -> mu*rstd
    nc.vector.tensor_scalar_mul(va[:], pmean[:], 1.0 / H)
    nc.vector.tensor_scalar_mul(vb[:], psq[:], 1.0 / H)
    nc.vector.tensor_tensor(out=vc[:], in0=va[:], in1=va[:], op=ALU.mult)
    nc.vector.tensor_tensor(out=vb[:], in0=vb[:], in1=vc[:], op=ALU.subtract)
    nc.scalar.activation(vc[:], vb[:], AF.Sqrt, bias=e["eps_t"][0:1, 0:1])
    vec2 = e["p_vec2"].tile([1, 2 * T], F32, tag="vec2")
    rstd, nmr = vec2[:, 0:T], vec2[:, T:2 * T]
    nc.vector.reciprocal(rstd, vc[:])
    nc.vector.tensor_tensor(out=vc[:], in0=va[:], in1=rstd, op=ALU.mult)
    nc.vector.tensor_scalar_mul(nmr, vc[:], -1.0)
    # broadcast rstd and -mu*rstd across partitions via a DRAM bounce
    dscr = e["p_dram"].tile([1, 2 * T], F32, tag="lnscr")
    nc.sync.dma_start(dscr[:], vec2[:])
    bc = e["p_lnbc"].tile([128, 2 * T], F32, tag="lnbc")
    nc.sync.dma_start(bc[:], dscr[:].partition_broadcast(128)[:, 0, :])
    out = p_xt.tile([128, HC, T], F32R, tag="xt")
    for k in range(HC):
        t2 = p_scr.tile([128, T], F32, tag="scr")
        nc.vector.tensor_tensor(out=t2[:], in0=A[:, k, :].bitcast(F32),
                                in1=bc[:, 0:T], op=ALU.mult)
        nc.vector.tensor_tensor(out=t2[:], in0=t2[:], in1=bc[:, T:2 * T], op=ALU.add)
        nc.scalar.activation(out[:, k, :], t2[:], AF.Identity,
                             scale=ln_sb[:, k:k + 1], bias=ln_sb[:, HC + k:HC + k + 1])
    return out


def _head(nc, tc, d, X, e):
    p_qk, p_f1, p_f2a, p_wi, p_lnbc = e["p_qk"], e["p_f1"], e["p_f2a"], e["p_wi"], e["p_lnbc"]
    p_scr, p_bias, p_sm = e["p_scr"], e["p_bias"], e["p_sm"]
    ps_mm = e["ps_mm"]
    pmat, out_d = e["pmat"], e["out_d"]

    # relu(x) transposed, bf16
    reluT = p_qk.tile([128, HC, T], BF16, tag="qk")
    for k in range(HC):
        nc.scalar.activation(reluT[:, k, :], X[:, k, :].bitcast(F32), AF.Relu)
    # f1 = relu(relu(x) @ w1 + b1), transposed layout [M1C, T]
    b1_t = p_bias.tile([128, M1C], F32, tag="bias")
    nc.sync.dma_start(b1_t[:], d["b1"].rearrange("(c p) -> p c", p=128))
    w1_t = p_f2a.tile([128, HC, M1], BF16, tag="f2a")
    nc.gpsimd.dma_start(w1_t[:], d["w1"].rearrange("(k p) f -> p k f", p=128))
    f1 = p_f1.tile([128, M1C, T], BF16, tag="f1")
    for m in range(M1C):
        pm_ = ps_mm.tile([128, T], F32, tag="ps_mm")
        for k in range(HC):
            nc.tensor.matmul(pm_[:], w1_t[:, k, 128 * m:128 * (m + 1)], reluT[:, k, :],
                             start=(k == 0), stop=(k == HC - 1))
        nc.scalar.activation(f1[:, m, :], pm_[:], AF.Relu, bias=b1_t[:, m:m + 1])
    # f2 = f1 @ w2 + b2, token-major [TC, C]
    w2_t = p_wi.tile([128, M1C, C], BF16, tag="wi")
    nc.gpsimd.dma_start(w2_t[:], d["w2"].rearrange("(k p) f -> p k f", p=128))
    b2bc = p_lnbc.tile([128, C], F32, tag="lnbc")
    nc.sync.dma_start(b2bc[:], d["b2"][None, :].partition_broadcast(128)[:, 0, :])
    f2 = p_f2a.tile([128, TC, CPAD], F32R, tag="f2a")
    nc.gpsimd.memset(f2[:].bitcast(F32), 0.0)
    for c in range(TC):
        pm_ = ps_mm.tile([128, C], F32, tag="ps_mm")
        for k in range(M1C):
            nc.tensor.matmul(pm_[:], f1[:, k, 128 * c:128 * (c + 1)], w2_t[:, k, :],
                             start=(k == 0), stop=(k == M1C - 1))
        nc.vector.tensor_tensor(out=f2[:, c, 0:C], in0=pm_[:], in1=b2bc[:],
                                op=ALU.add)

    # pooling + final softmax (N padded to 428 for fp32r)
    CP2 = 428
    for s in range(BPC):
        ppool = ps_mm.tile([128, CP2], F32, tag="ps_mm")
        for j in range(2):
            pm_t = p_sm.tile([128, 128], F32R, tag="pm")
            nc.sync.dma_start(pm_t[:], pmat[256 * s + 128 * j:256 * s + 128 * (j + 1), :].bitcast(F32R))
            nc.tensor.matmul(ppool[:], pm_t[:], f2[:, 2 * s + j, 0:CP2],
                             start=(j == 0), stop=(j == 1))
        for half, src in ((0, ppool[:, 0:C]), (1, f2[:, 2 * s + 1, 0:C].bitcast(F32))):
            ex = p_scr.tile([128, CPAD], F32, tag="scr")
            se = p_sm.tile([128, 2], F32, tag="se")
            nc.scalar.activation(ex[:, 0:C], src, AF.Exp, accum_out=se[:, 0:1])
            nc.vector.reciprocal(se[:, 1:2], se[:, 0:1])
            nc.vector.tensor_scalar_mul(ex[:, 0:C], ex[:, 0:C], se[:, 1:2])
            row0 = 256 * s + 128 * half
            nc.sync.dma_start(out_d[row0:row0 + 128, :], ex[:, 0:C])


# ======================= host side =======================

_PROG_CACHE = {}


def _get_program(n_layers=L):
    if n_layers not in _PROG_CACHE:
        _PROG_CACHE[n_layers] = build_program(n_layers)
    return _PROG_CACHE[n_layers]


def make_in_maps(inputs, n_layers=L):
    """Build per-core input maps from the full-problem inputs dict."""
    f32 = lambda x: np.ascontiguousarray(np.asarray(x), dtype=np.float32)
    enc = np.asarray(inputs["encoded_batch"], dtype=np.int32)
    mask = np.asarray(inputs["mask"], dtype=np.int32)
    wpt = np.asarray(inputs["word_piece_tracked"], dtype=np.int32)

    # pooling matrix P[b, s, w] = 1/cnt[b,w] if seg[b,s]==w else 0
    cum = np.cumsum(wpt, axis=1)                      # [B, W]
    P = np.zeros((B, S, W), dtype=np.float32)
    for b in range(B):
        seg = np.searchsorted(cum[b], np.arange(S), side="right")  # [S]
        valid = seg < W
        P[b, np.arange(S)[valid], seg[valid]] = 1.0 / wpt[b, seg[valid]]

    ab = (1.0 - mask.astype(np.float32)) * -10000.0   # [B, S]

    rep = {}
    for k in ["word_emb", "pos_emb", "type_emb", "emb_ln_s", "emb_ln_b",
              "Wq", "bq", "Wk", "bk", "Wv", "bv", "Wo", "bo", "ln1_s", "ln1_b",
              "Wi", "bi", "Wo2", "bo2", "ln2_s", "ln2_b", "w1", "b1", "w2", "b2"]:
        rep[k] = f32(inputs[k])
    rep["cones"] = np.ones((128, 1), dtype=np.float32)
    rep["ceps"] = np.full((128, 1), EPS, dtype=np.float32)
    rep["crow"] = np.ones((1, 128), dtype=np.float32)

    in_maps = []
    for core in range(N_CORES):
        b0 = core * BPC
        m = dict(rep)
        m["enc"] = enc[b0:b0 + BPC].reshape(T, 1)
        m["ab"] = ab[b0:b0 + BPC].reshape(T)
        m["pmat"] = P[b0:b0 + BPC].reshape(T, W)
        in_maps.append(m)
    return in_maps


def kernel(**inputs):
    nc = _get_program(L)
    in_maps = make_in_maps(inputs, L)
    res = run_bass_kernel_spmd(nc, in_maps, core_ids=list(range(N_CORES)))
    out = np.concatenate([res.results[i]["out"].reshape(BPC, S, C)
                          for i in range(N_CORES)], axis=0)
    return out.astype(np.float32)



# revision 20
# speedup vs baseline: 1.0767x; 1.0767x over previous
"""Trainium2 Bass kernel for the CCG supertagger BERT model.

Data-parallel over batch: 16 samples -> 8 cores x 2 samples.
Activations transposed [H (6 chunks of 128), T=512 tokens] in SBUF.
Host pre-casts weights to bf16 (optionally fp8e4 x WS for QKV/FFN with
DoubleRow matmuls). LayerNorm stats broadcast via PE ones-matmul
(no DRAM bounce); softmax denominators via a ones column in V.
"""
import os
import numpy as np
import ml_dtypes

import concourse.bass as bass
import concourse.tile as tile
from concourse import bacc, mybir
from concourse.bass_utils import run_bass_kernel_spmd
from concourse.masks import make_identity

F32 = mybir.dt.float32
F32R = mybir.dt.float32r
BF16 = mybir.dt.bfloat16
FP8 = mybir.dt.float8e4
I32 = mybir.dt.int32
AF = mybir.ActivationFunctionType
ALU = mybir.AluOpType
DR = mybir.MatmulPerfMode.DoubleRow
# CoreSim has no Gelu; BASS_SIM_TANH=1 swaps in Tanh for sim-only checks
AF_FF = (mybir.ActivationFunctionType.Tanh if os.environ.get("BASS_SIM_TANH")
         else mybir.ActivationFunctionType.Gelu)

B, S, W = 16, 256, 128
V, H, L, NH, DH, FF, C = 30522, 768, 12, 12, 64, 3072, 425
EPS = 1e-12
N_CORES = 8
BPC = B // N_CORES          # samples per core
T = BPC * S                 # tokens per core (512)
HC = H // 128               # 6
FFC = FF // 128             # 24
TC = T // 128               # 4 token chunks
M1 = 1024
M1C = M1 // 128             # 8
CPAD = 448                  # padded class dim for sbuf tiles
VW = 65                     # per-head V width incl. ones column

FP8_QKV = os.environ.get("BASS_FP8_QKV", "0") == "1"   # Wq/Wk/Wv DoubleRow
FP8_FFN = os.environ.get("BASS_FP8_FFN", "0") == "1"   # Wi / Wo2 DoubleRow
WS = 64.0                   # fp8 weight pre-scale

MOV_QKV = FP8 if FP8_QKV else BF16    # dtype of X moving copy
MOV_FFN = FP8 if FP8_FFN else BF16    # dtype of H2 moving copy
GEL_DT = FP8 if FP8_FFN else BF16     # gelu tile dtype


def build_program(n_layers=L):
    nc = bacc.Bacc("TRN2", target_bir_lowering=False, debug=False,
                   num_devices=N_CORES)

    dt_ = lambda name, shape, dt, kind="ExternalInput": nc.dram_tensor(
        name, shape, dt, kind=kind).ap()
    wdt_qkv = FP8 if FP8_QKV else BF16
    wdt_ffn = FP8 if FP8_FFN else BF16
    d = dict(
        enc=dt_("enc", [T, 1], I32),
        ab=dt_("ab", [T], F32),
        pmat=dt_("pmat", [T, 128], F32),
        word_emb=dt_("word_emb", [V, H], F32),
        pos2=dt_("pos2", [S, H], BF16),          # pos_emb + type_emb
        emb_ln_s=dt_("emb_ln_s", [H], F32),
        emb_ln_b=dt_("emb_ln_b", [H], F32),
        Wq=dt_("Wq", [L, H, H], wdt_qkv),
        Wk=dt_("Wk", [L, H, H], wdt_qkv),
        Wv=dt_("Wv", [L, H, H], wdt_qkv),
        Wo=dt_("Wo", [L, H, H], BF16),
        bq=dt_("bq", [L, H], F32), bk=dt_("bk", [L, H], F32),
        bv=dt_("bv", [L, H], F32), bo=dt_("bo", [L, H], F32),
        ln1_s=dt_("ln1_s", [L, H], F32), ln1_b=dt_("ln1_b", [L, H], F32),
        Wi=dt_("Wi", [L, H, FF], wdt_ffn),
        bi=dt_("bi", [L, FF], F32),
        Wo2=dt_("Wo2", [L, FF, H], wdt_ffn),
        bo2=dt_("bo2", [L, H], F32),
        bo2r=dt_("bo2r", [L, 1, H], BF16),       # WS*bo2 row (fp8 path)
        ln2_s=dt_("ln2_s", [L, H], F32), ln2_b=dt_("ln2_b", [L, H], F32),
        w1=dt_("w1", [H, M1], BF16), b1=dt_("b1", [M1], F32),
        w2=dt_("w2", [M1, C], BF16), b2=dt_("b2", [C], F32),
        cones=dt_("cones", [128, 1], F32),       # value 1/H
        crow=dt_("crow", [1, 128], F32),         # ones row
        ceps=dt_("ceps", [128, 1], F32),         # EPS
        out_d=dt_("out", [T, C], F32, "ExternalOutput"),
    )

    with tile.TileContext(nc) as tc:
        _emit(nc, tc, n_layers, d)
    nc.compile()
    return nc


def _emit(nc, tc, n_layers, d):
    from contextlib import ExitStack
    ctx = ExitStack()
    with ctx:
        _emit_body(nc, tc, n_layers, d, ctx)


def _emit_body(nc, tc, n_layers, d, ctx):
    pool = lambda name, bufs, space="SBUF": ctx.enter_context(
        tc.tile_pool(name=name, bufs=bufs, space=space))

    p_x = pool("x", 2)            # [128, HC, 512] f32r residual (in-place adds)
    p_xb = pool("xb", 2)          # [128, HC, 512] bf16/fp8 moving copies
    p_qk = pool("qk", 3)          # [128, HC, 512] bf16 QT/KT/VT/ctxT/reluT
    p_v = pool("v", 1)            # [128, TC, 780] bf16 token-major V + ones
    p_ctx = pool("ctxp", 1)       # [128, TC, 768] bf16 token-major ctx / f2
    p_exp = pool("exp", 2)        # [128, 2, 256] bf16 exp tiles
    p_scr = pool("scr", 2)        # [128, 512] f32 scratch (sq, t1, t2)
    p_gel = pool("gel", 1)        # [128, FFC, 512] gelu / emb x0 / head f1
    p_w6 = pool("w6", 2)          # [128, HC, 768] QKVO weights / head w2
    p_wi = pool("wi", 1)          # [128, HC, 3072] Wi / head w1
    p_wo2 = pool("wo2", 1)        # [128, FFC, 768] Wo2
    p_bias = pool("bias", 6)      # [128, <=48] f32 bias/scale tiles
    p_vec = pool("vec", 2)        # [1, 512] f32 LN stat vectors
    p_sm = pool("sm", 3)          # small scalars / pooling matrices
    p_cst = pool("cst", 1)        # constants
    p_pos = pool("pos", 1)        # [128, 2, H] pos emb / head b2bc

    ps_mm = pool("ps_mm", 3, "PSUM")    # [128, 512] f32 matmul groups
    ps_sc = pool("ps_sc", 2, "PSUM")    # [128, 256] scores / [1, 512] stats
    ps_cx = pool("ps_cx", 3, "PSUM")    # [128, 65] ctx groups

    enc, ab, pmat, out_d = d["enc"], d["ab"], d["pmat"], d["out_d"]

    # ---- constants ----
    ident = p_cst.tile([128, 128], BF16, tag="ident")
    make_identity(nc, ident[:])
    identf = p_cst.tile([128, 128], F32, tag="identf")
    make_identity(nc, identf[:])
    ones_c = p_cst.tile([128, 1], F32R, tag="ones_c")       # value 1/H
    nc.sync.dma_start(ones_c[:], d["cones"][:].bitcast(F32R))
    crow = p_cst.tile([1, 128], F32R, tag="crow")           # bcast lhsT
    nc.sync.dma_start(crow[:], d["crow"][:].bitcast(F32R))
    eps_t = p_cst.tile([128, 1], F32, tag="eps")
    nc.sync.dma_start(eps_t[:], d["ceps"][:])
    ones_T = p_cst.tile([1, T], BF16, tag="onesT")          # bias-row rhs
    nc.vector.memset(ones_T[:], 1.0)

    ab_t = p_cst.tile([128, TC], F32, tag="ab")
    nc.sync.dma_start(ab_t[:], ab.rearrange("(c p) -> p c", p=128))

    def ln_pair(ap_s, ap_b, tag):
        t = p_bias.tile([128, 2 * HC], F32, tag="bias")
        nc.sync.dma_start(t[:, 0:HC], ap_s.rearrange("(c p) -> p c", p=128))
        nc.sync.dma_start(t[:, HC:2 * HC], ap_b.rearrange("(c p) -> p c", p=128))
        return t

    def wload(pl, tag, src2d, nk, ncols, dt):
        t = pl.tile([128, nk, ncols], dt, tag=tag)
        nc.gpsimd.dma_start(t[:], src2d.rearrange("(k p) f -> p k f", p=128))
        return t

    e = dict(p_x=p_x, p_xb=p_xb, p_qk=p_qk, p_v=p_v, p_ctx=p_ctx, p_exp=p_exp,
             p_scr=p_scr, p_gel=p_gel, p_w6=p_w6, p_wi=p_wi, p_wo2=p_wo2,
             p_bias=p_bias, p_vec=p_vec, p_sm=p_sm,
             ps_mm=ps_mm, ps_sc=ps_sc, ps_cx=ps_cx,
             ones_c=ones_c, crow=crow, eps_t=eps_t, ones_T=ones_T, ab_t=ab_t,
             ident=ident, ln_pair=ln_pair, wload=wload, d=d)

    # =============== embedding (bf16 token-major, activation normalize) ===
    x0 = p_gel.tile([128, TC, H], BF16, tag="gel")
    for c in range(TC):
        idx_t = p_sm.tile([128, 1], I32, tag="idx")
        nc.sync.dma_start(idx_t[:], enc[128 * c:128 * (c + 1), :])
        nc.gpsimd.indirect_dma_start(
            out=x0[:, c, :], out_offset=None, in_=d["word_emb"][:],
            in_offset=bass.IndirectOffsetOnAxis(ap=idx_t[:, :1], axis=0))
    pos_t = p_pos.tile([128, 2, H], BF16, tag="pos")
    nc.sync.dma_start(pos_t[:, 0, :], d["pos2"][0:128, :])
    nc.sync.dma_start(pos_t[:, 1, :], d["pos2"][128:256, :])

    emb_sb = ln_pair(d["emb_ln_s"], d["emb_ln_b"], "embln")
    X = p_x.tile([128, HC, T], F32R, tag="x")
    Xb = p_xb.tile([128, HC, T], MOV_QKV, tag="xb")
    for c in range(TC):
        xc = x0[:, c, :]
        nc.vector.tensor_tensor(out=xc, in0=xc, in1=pos_t[:, c % 2, :], op=ALU.add)
        su = p_sm.tile([128, 4], F32, tag="stat")
        nc.vector.reduce_sum(out=su[:, 0:1], in_=xc, axis=mybir.AxisListType.X)
        sq = p_scr.tile([128, H], F32, tag="scr")
        nc.scalar.activation(sq[:], xc, AF.Square, accum_out=su[:, 1:2])
        st = p_sm.tile([128, 4], F32, tag="stat2")
        nc.vector.tensor_scalar_mul(st[:, 0:1], su[:, 0:1], 1.0 / H)      # mu
        nc.vector.tensor_scalar_mul(st[:, 1:2], su[:, 1:2], 1.0 / H)      # m2
        nc.vector.tensor_tensor(out=st[:, 2:3], in0=st[:, 0:1], in1=st[:, 0:1], op=ALU.mult)
        nc.vector.tensor_tensor(out=st[:, 3:4], in0=st[:, 1:2], in1=st[:, 2:3], op=ALU.subtract)
        sd = p_sm.tile([128, 4], F32, tag="stat3")
        nc.scalar.activation(sd[:, 0:1], st[:, 3:4], AF.Sqrt, bias=eps_t[:, 0:1])
        nc.vector.reciprocal(sd[:, 1:2], sd[:, 0:1])
        # bias = -mu*rstd so normalize folds into one activation per chunk
        nc.vector.tensor_tensor(out=sd[:, 2:3], in0=st[:, 0:1], in1=sd[:, 1:2], op=ALU.mult)
        nc.vector.tensor_scalar_mul(sd[:, 3:4], sd[:, 2:3], -1.0)
        nc.scalar.activation(xc, xc, AF.Identity, scale=sd[:, 1:2], bias=sd[:, 3:4])
        for k in range(HC):
            pt = ps_mm.tile([128, 128], BF16, tag="ps_mm")
            nc.tensor.transpose(pt[:], xc[:, 128 * k:128 * (k + 1)], ident[:])
            nc.scalar.activation(X[:, k, 128 * c:128 * (c + 1)], pt[:],
                                 AF.Identity, scale=emb_sb[:, k:k + 1],
                                 bias=emb_sb[:, HC + k:HC + k + 1])
            nc.scalar.activation(Xb[:, k, 128 * c:128 * (c + 1)], pt[:],
                                 AF.Identity, scale=emb_sb[:, k:k + 1],
                                 bias=emb_sb[:, HC + k:HC + k + 1])

    # =============== transformer layers ===============
    for l in range(n_layers):
        X, Xb = _layer(nc, tc, l, X, Xb, e)

    # =============== head ===============
    _head(nc, tc, X, e, out_d, pmat, p_pos)


def _mm_group(nc, pm, wt, mov, m, fp8, nk=HC):
    """Accumulate one output block: psum[128,T] += W[:,k,128m:...]T @ mov."""
    if fp8:
        np_ = nk // 2
        for jp in range(np_):
            nc.tensor.matmul(pm[:], wt[:, 2 * jp:2 * jp + 2, 128 * m:128 * (m + 1)],
                             mov[:, 2 * jp:2 * jp + 2, :],
                             start=(jp == 0), stop=(jp == np_ - 1), perf_mode=DR)
    else:
        for k in range(nk):
            nc.tensor.matmul(pm[:], wt[:, k, 128 * m:128 * (m + 1)], mov[:, k, :],
                             start=(k == 0), stop=(k == nk - 1))


def _ln_new(nc, e, A, pmean, psq, sb, out_mov_dt):
    """LayerNorm over partition(H) dim. A: [128,HC,T] f32r; pmean/psq: [1,T]
    psum tiles holding sum(x)/H and sum(x^2)/H. Returns (X_f32, X_mov)."""
    p_vec, p_scr, p_x, p_xb, ps_mm, crow, eps_t = (
        e["p_vec"], e["p_scr"], e["p_x"], e["p_xb"], e["ps_mm"], e["crow"], e["eps_t"])
    v1 = p_vec.tile([1, T], F32, tag="vec")     # mu^2
    nc.scalar.activation(v1[:], pmean[:], AF.Square)
    v2 = p_vec.tile([1, T], F32, tag="vec")     # var
    nc.vector.tensor_tensor(out=v2[:], in0=psq[:], in1=v1[:], op=ALU.subtract)
    v2e = p_vec.tile([1, T], F32, tag="vec")    # var + eps
    nc.vector.tensor_scalar_add(v2e[:], v2[:], EPS)
    vr = p_vec.tile([1, T], F32, tag="vec")     # 1/(var+eps)
    nc.vector.reciprocal_approx_fast(vr[:], v2e[:])
    v3 = p_vec.tile([1, T], F32R, tag="vec")    # rstd (f32r for bcast matmul)
    nc.scalar.activation(v3[:], vr[:], AF.Sqrt)
    v4 = p_vec.tile([1, T], F32R, tag="vec")    # mu*rstd
    nc.vector.tensor_tensor(out=v4[:], in0=pmean[:], in1=v3[:].bitcast(F32), op=ALU.mult)
    bc_r = ps_mm.tile([128, T], F32, tag="ps_mm")
    nc.tensor.matmul(bc_r[:], crow[:], v3[:], start=True, stop=True)
    bc_m = ps_mm.tile([128, T], F32, tag="ps_mm")
    nc.tensor.matmul(bc_m[:], crow[:], v4[:], start=True, stop=True)
    Xn = p_x.tile([128, HC, T], F32R, tag="x")
    Xm = p_xb.tile([128, HC, T], out_mov_dt, tag="xb")
    for k in range(HC):
        t1 = p_scr.tile([128, T], F32, tag="scr")
        nc.vector.tensor_tensor(out=t1[:], in0=A[:, k, :].bitcast(F32),
                                in1=bc_r[:], op=ALU.mult)
        t2 = p_scr.tile([128, T], F32, tag="scr")
        nc.vector.tensor_tensor(out=t2[:], in0=t1[:], in1=bc_m[:], op=ALU.subtract)
        nc.scalar.activation(Xm[:, k, :], t2[:], AF.Identity,
                             scale=sb[:, k:k + 1], bias=sb[:, HC + k:HC + k + 1])
        nc.vector.tensor_scalar(out=Xn[:, k, :], in0=t2[:],
                                scalar1=sb[:, k:k + 1], scalar2=sb[:, HC + k:HC + k + 1],
                                op0=ALU.mult, op1=ALU.add)
    return Xn, Xm


def _stat_mms(nc, e, A, k, pmean, psq):
    """Emit LN stat matmuls for chunk k of A (accumulating [1,T] psums)."""
    sq = e["p_scr"].tile([128, T], F32R, tag="scr")
    nc.scalar.activation(sq[:], A[:, k, :].bitcast(F32), AF.Square)
    nc.tensor.matmul(pmean[:], e["ones_c"][:], A[:, k, :],
                     start=(k == 0), stop=(k == HC - 1))
    nc.tensor.matmul(psq[:], e["ones_c"][:], sq[:],
                     start=(k == 0), stop=(k == HC - 1))


def _layer(nc, tc, l, X, Xb, e):
    mov_qkv = Xb
    d = e["d"]
    p_qk, p_v, p_ctx, p_exp = e["p_qk"], e["p_v"], e["p_ctx"], e["p_exp"]
    p_scr, p_gel, p_w6, p_wi, p_wo2 = e["p_scr"], e["p_gel"], e["p_w6"], e["p_wi"], e["p_wo2"]
    p_bias, p_sm, p_x = e["p_bias"], e["p_sm"], e["p_x"]
    ps_mm, ps_sc, ps_cx = e["ps_mm"], e["ps_sc"], e["ps_cx"]
    ident, ab_t, ones_T = e["ident"], e["ab_t"], e["ones_T"]
    ln_pair, wload = e["ln_pair"], e["wload"]

    # single-buffered big FFN weights: DMA fires once prev layer's use ends
    wi_t = wload(p_wi, "wi", d["Wi"][l], HC, FF, FP8 if FP8_FFN else BF16)
    wo2_t = wload(p_wo2, "wo2", d["Wo2"][l], FFC, H, FP8 if FP8_FFN else BF16)

    qkvo_b = p_bias.tile([128, 4 * HC], F32, tag="bias")
    for i, bap in enumerate([d["bq"], d["bk"], d["bv"], d["bo"]]):
        nc.sync.dma_start(qkvo_b[:, i * HC:(i + 1) * HC],
                          bap[l].rearrange("(c p) -> p c", p=128))

    wdt_qkv = FP8 if FP8_QKV else BF16
    # ---- Q, K projections (interleaved m-blocks so attention starts early)
    QT = p_qk.tile([128, HC, T], BF16, tag="qk")
    KT = p_qk.tile([128, HC, T], BF16, tag="qk")
    wq_t = wload(p_w6, "w6", d["Wq"][l], HC, H, wdt_qkv)
    wk_t = wload(p_w6, "w6", d["Wk"][l], HC, H, wdt_qkv)
    for m in range(HC):
        for wt, dst, boff in ((wq_t, QT, 0), (wk_t, KT, HC)):
            pm = ps_mm.tile([128, T], F32, tag="ps_mm")
            _mm_group(nc, pm, wt, mov_qkv, m, FP8_QKV)
            nc.scalar.activation(dst[:, m, :], pm[:], AF.Identity,
                                 scale=(1.0 / WS if FP8_QKV else 1.0),
                                 bias=qkvo_b[:, boff + m:boff + m + 1])
    # ---- V projection + transpose to token-major (with ones columns)
    VT = p_qk.tile([128, HC, T], BF16, tag="qk")
    wv_t = wload(p_w6, "w6", d["Wv"][l], HC, H, wdt_qkv)
    for m in range(HC):
        pm = ps_mm.tile([128, T], F32, tag="ps_mm")
        _mm_group(nc, pm, wv_t, mov_qkv, m, FP8_QKV)
        if FP8_QKV:
            nc.vector.tensor_scalar(out=VT[:, m, :], in0=pm[:], scalar1=1.0 / WS,
                                    scalar2=qkvo_b[:, 2 * HC + m:2 * HC + m + 1],
                                    op0=ALU.mult, op1=ALU.add)
        else:
            nc.vector.tensor_scalar_add(VT[:, m, :], pm[:],
                                        qkvo_b[:, 2 * HC + m:2 * HC + m + 1])
    Vtok = p_v.tile([128, TC, NH * VW], BF16, tag="v")
    for h in range(NH):
        nc.gpsimd.memset(Vtok[:, :, VW * h + 64:VW * h + 65], 1.0)
    for c in range(TC):
        for k in range(HC):
            pt = ps_mm.tile([128, 128], BF16, tag="ps_mm")
            nc.tensor.transpose(pt[:], VT[:, k, 128 * c:128 * (c + 1)], ident[:])
            for hh in range(2):            # head-aligned 64-col halves
                h = 2 * k + hh
                nc.scalar.activation(Vtok[:, c, VW * h:VW * h + 64],
                                     pt[:, 64 * hh:64 * hh + 64], AF.Copy)

    # ---- attention (h-major; ctxT chunk k emitted after heads 2k,2k+1) ----
    ctok = p_ctx.tile([128, TC, H], BF16, tag="ctxp")
    ctxT = p_qk.tile([128, HC, T], BF16, tag="qk")
    for h in range(NH):
        kc, po = h // 2, 64 * (h % 2)
        for s in range(BPC):
            expt = p_exp.tile([128, 2, 256], BF16, tag="exp")
            for j in range(2):
                psc = ps_sc.tile([128, 256], F32, tag="ps_sc")
                nc.tensor.matmul(
                    psc[:],
                    KT[po:po + 64, kc, 256 * s + 128 * j:256 * s + 128 * (j + 1)],
                    QT[po:po + 64, kc, 256 * s:256 * (s + 1)],
                    start=True, stop=True)
                nc.scalar.activation(expt[:, j, :], psc[:], AF.Exp, scale=0.125,
                                     bias=ab_t[:, 2 * s + j:2 * s + j + 1])
            for i in range(2):
                pcx = ps_cx.tile([128, VW], F32, tag="ps_cx")
                for j in range(2):
                    nc.tensor.matmul(pcx[:], expt[:, j, 128 * i:128 * (i + 1)],
                                     Vtok[:, 2 * s + j, VW * h:VW * (h + 1)],
                                     start=(j == 0), stop=(j == 1))
                rec = p_sm.tile([128, 1], F32, tag="rec")
                nc.vector.reciprocal(rec[:], pcx[:, 64:65])
                nc.vector.tensor_scalar_mul(
                    ctok[:, 2 * s + i, 64 * h:64 * h + 64], pcx[:, 0:64], rec[:])
        if h % 2 == 1:
            k = h // 2
            for c in range(TC):
                pt = ps_mm.tile([128, 128], BF16, tag="ps_mm")
                nc.tensor.transpose(pt[:], ctok[:, c, 128 * k:128 * (k + 1)], ident[:])
                nc.vector.tensor_copy(ctxT[:, k, 128 * c:128 * (c + 1)], pt[:])

    # ---- O-projection + residual; LN1 stats pipelined per chunk ----
    ln1 = ln_pair(d["ln1_s"][l], d["ln1_b"][l], "ln1")
    wo_t = wload(p_w6, "w6", d["Wo"][l], HC, H, BF16)
    A = X                                  # in-place residual add
    pmean1 = ps_sc.tile([1, T], F32, tag="ps_sc")
    psq1 = ps_sc.tile([1, T], F32, tag="ps_sc")
    for m in range(HC):
        pm = ps_mm.tile([128, T], F32, tag="ps_mm")
        _mm_group(nc, pm, wo_t, ctxT, m, False)
        nc.vector.scalar_tensor_tensor(
            out=A[:, m, :], in0=pm[:],
            scalar=qkvo_b[:, 3 * HC + m:3 * HC + m + 1],
            in1=X[:, m, :].bitcast(F32), op0=ALU.add, op1=ALU.add)
        _stat_mms(nc, e, A, m, pmean1, psq1)
    H2, H2b = _ln_new(nc, e, A, pmean1, psq1, ln1, MOV_FFN)
    mov_ffn = H2b

    # ---- FFN: gelu all FFC chunks, then PSUM-accumulated down-proj ----
    ln2 = ln_pair(d["ln2_s"][l], d["ln2_b"][l], "ln2")
    bi_t = p_bias.tile([128, FFC], F32, tag="bias")
    nc.sync.dma_start(bi_t[:], d["bi"][l].rearrange("(c p) -> p c", p=128))
    gel = p_gel.tile([128, FFC, T], GEL_DT, tag="gel")
    for m in range(FFC):
        pm = ps_mm.tile([128, T], F32, tag="ps_mm")
        _mm_group(nc, pm, wi_t, mov_ffn, m, FP8_FFN)
        nc.scalar.activation(gel[:, m, :], pm[:], AF_FF,
                             scale=(1.0 / WS if FP8_FFN else 1.0),
                             bias=bi_t[:, m:m + 1])
    if FP8_FFN:
        bo2r = e["p_vec"].tile([1, H], BF16, tag="vec")
        nc.sync.dma_start(bo2r[:], d["bo2r"][l])
    else:
        bo2_t = p_bias.tile([128, HC], F32, tag="bias")
        nc.sync.dma_start(bo2_t[:], d["bo2"][l].rearrange("(c p) -> p c", p=128))
    Apre = H2                              # in-place residual add
    pmean2 = ps_sc.tile([1, T], F32, tag="ps_sc")
    psq2 = ps_sc.tile([1, T], F32, tag="ps_sc")
    for o in range(HC):
        pm = ps_mm.tile([128, T], F32, tag="ps_mm")
        if FP8_FFN:
            npr = FFC // 2
            for jp in range(npr):
                nc.tensor.matmul(pm[:], wo2_t[:, 2 * jp:2 * jp + 2, 128 * o:128 * (o + 1)],
                                 gel[:, 2 * jp:2 * jp + 2, :],
                                 start=(jp == 0), stop=False, perf_mode=DR)
            nc.tensor.matmul(pm[:], bo2r[0:1, 128 * o:128 * (o + 1)], ones_T[:],
                             start=False, stop=True)
            nc.vector.scalar_tensor_tensor(
                out=Apre[:, o, :], in0=pm[:], scalar=1.0 / WS,
                in1=H2[:, o, :].bitcast(F32), op0=ALU.mult, op1=ALU.add)
        else:
            for k in range(FFC):
                nc.tensor.matmul(pm[:], wo2_t[:, k, 128 * o:128 * (o + 1)],
                                 gel[:, k, :], start=(k == 0), stop=(k == FFC - 1))
            nc.vector.scalar_tensor_tensor(
                out=Apre[:, o, :], in0=pm[:],
                scalar=bo2_t[:, o:o + 1],
                in1=H2[:, o, :].bitcast(F32), op0=ALU.add, op1=ALU.add)
        _stat_mms(nc, e, Apre, o, pmean2, psq2)
    return _ln_new(nc, e, Apre, pmean2, psq2, ln2, MOV_QKV)


def _head(nc, tc, X, e, out_d, pmat, p_pos):
    d = e["d"]
    p_qk, p_gel, p_wi, p_w6, p_ctx = e["p_qk"], e["p_gel"], e["p_wi"], e["p_w6"], e["p_ctx"]
    p_scr, p_bias, p_sm = e["p_scr"], e["p_bias"], e["p_sm"]
    ps_mm = e["ps_mm"]

    reluT = p_qk.tile([128, HC, T], BF16, tag="qk")
    for k in range(HC):
        nc.scalar.activation(reluT[:, k, :], X[:, k, :].bitcast(F32), AF.Relu)
    b1_t = p_bias.tile([128, M1C], F32, tag="bias")
    nc.sync.dma_start(b1_t[:], d["b1"].rearrange("(c p) -> p c", p=128))
    w1_t = p_wi.tile([128, HC, M1], BF16, tag="wi")
    nc.gpsimd.dma_start(w1_t[:], d["w1"].rearrange("(k p) f -> p k f", p=128))
    f1 = p_gel.tile([128, M1C, T], BF16, tag="gel")
    for m in range(M1C):
        pm = ps_mm.tile([128, T], F32, tag="ps_mm")
        for k in range(HC):
            nc.tensor.matmul(pm[:], w1_t[:, k, 128 * m:128 * (m + 1)], reluT[:, k, :],
                             start=(k == 0), stop=(k == HC - 1))
        nc.scalar.activation(f1[:, m, :], pm[:], AF.Relu, bias=b1_t[:, m:m + 1])
    w2_t = p_w6.tile([128, M1C, C], BF16, tag="w6")
    nc.gpsimd.dma_start(w2_t[:], d["w2"].rearrange("(k p) f -> p k f", p=128))
    b2bc = p_pos.tile([128, C], F32, tag="pos")
    nc.sync.dma_start(b2bc[:], d["b2"][None, :].partition_broadcast(128)[:, 0, :])
    f2 = p_ctx.tile([128, TC, CPAD], F32R, tag="ctxp")
    nc.gpsimd.memset(f2[:].bitcast(F32), 0.0)
    for c in range(TC):
        pm = ps_mm.tile([128, C], F32, tag="ps_mm")
        for k in range(M1C):
            nc.tensor.matmul(pm[:], f1[:, k, 128 * c:128 * (c + 1)], w2_t[:, k, :],
                             start=(k == 0), stop=(k == M1C - 1))
        nc.vector.tensor_tensor(out=f2[:, c, 0:C], in0=pm[:], in1=b2bc[:],
                                op=ALU.add)
    CP2 = 428
    for s in range(BPC):
        ppool = ps_mm.tile([128, CP2], F32, tag="ps_mm")
        for j in range(2):
            pm_t = p_sm.tile([128, 128], F32R, tag="pm")
            nc.sync.dma_start(pm_t[:], pmat[256 * s + 128 * j:256 * s + 128 * (j + 1), :].bitcast(F32R))
            nc.tensor.matmul(ppool[:], pm_t[:], f2[:, 2 * s + j, 0:CP2],
                             start=(j == 0), stop=(j == 1))
        for half, src in ((0, ppool[:, 0:C]), (1, f2[:, 2 * s + 1, 0:C].bitcast(F32))):
            ex = p_scr.tile([128, CPAD], F32, tag="scr")
            se = p_sm.tile([128, 2], F32, tag="se")
            nc.scalar.activation(ex[:, 0:C], src, AF.Exp, accum_out=se[:, 0:1])
            nc.vector.reciprocal(se[:, 1:2], se[:, 0:1])
            nc.vector.tensor_scalar_mul(ex[:, 0:C], ex[:, 0:C], se[:, 1:2])
            row0 = 256 * s + 128 * half
            nc.sync.dma_start(out_d[row0:row0 + 128, :], ex[:, 0:C])


# ======================= host side =======================

_PROG_CACHE = {}


def _get_program(n_layers=L):
    if n_layers not in _PROG_CACHE:
        _PROG_CACHE[n_layers] = build_program(n_layers)
    return _PROG_CACHE[n_layers]


def make_in_maps(inputs, n_layers=L):
    f32 = lambda x: np.ascontiguousarray(np.asarray(x), dtype=np.float32)
    bf16 = lambda x: np.ascontiguousarray(
        np.asarray(x, dtype=np.float32).astype(ml_dtypes.bfloat16))
    fp8w = lambda x: np.ascontiguousarray(
        (np.asarray(x, dtype=np.float32) * WS).astype(ml_dtypes.float8_e4m3fn))
    cvt_qkv = fp8w if FP8_QKV else bf16
    cvt_ffn = fp8w if FP8_FFN else bf16

    enc = np.asarray(inputs["encoded_batch"], dtype=np.int32)
    mask = np.asarray(inputs["mask"], dtype=np.int32)
    wpt = np.asarray(inputs["word_piece_tracked"], dtype=np.int32)

    cum = np.cumsum(wpt, axis=1)
    P = np.zeros((B, S, W), dtype=np.float32)
    for b in range(B):
        seg = np.searchsorted(cum[b], np.arange(S), side="right")
        valid = seg < W
        P[b, np.arange(S)[valid], seg[valid]] = 1.0 / wpt[b, seg[valid]]
    ab = (1.0 - mask.astype(np.float32)) * -10000.0

    rep = dict(
        word_emb=f32(inputs["word_emb"]),
        pos2=np.ascontiguousarray(
            (np.asarray(inputs["pos_emb"], np.float32)
             + np.asarray(inputs["type_emb"], np.float32)[0]
             ).astype(ml_dtypes.bfloat16)),
        emb_ln_s=f32(inputs["emb_ln_s"]), emb_ln_b=f32(inputs["emb_ln_b"]),
        Wq=cvt_qkv(inputs["Wq"]), Wk=cvt_qkv(inputs["Wk"]),
        Wv=cvt_qkv(inputs["Wv"]), Wo=bf16(inputs["Wo"]),
        bq=f32(inputs["bq"]), bk=f32(inputs["bk"]),
        bv=f32(inputs["bv"]), bo=f32(inputs["bo"]),
        ln1_s=f32(inputs["ln1_s"]), ln1_b=f32(inputs["ln1_b"]),
        Wi=cvt_ffn(inputs["Wi"]), bi=f32(inputs["bi"]),
        Wo2=cvt_ffn(inputs["Wo2"]), bo2=f32(inputs["bo2"]),
        bo2r=np.ascontiguousarray(
            (np.asarray(inputs["bo2"], np.float32) * WS)[:, None, :]
            .astype(ml_dtypes.bfloat16)),
        ln2_s=f32(inputs["ln2_s"]), ln2_b=f32(inputs["ln2_b"]),
        w1=bf16(inputs["w1"]), b1=f32(inputs["b1"]),
        w2=bf16(inputs["w2"]), b2=f32(inputs["b2"]),
        cones=np.full((128, 1), 1.0 / H, dtype=np.float32),
        crow=np.ones((1, 128), dtype=np.float32),
        ceps=np.full((128, 1), EPS, dtype=np.float32),
    )

    in_maps = []
    for core in range(N_CORES):
        b0 = core * BPC
        m = dict(rep)
        m["enc"] = enc[b0:b0 + BPC].reshape(T, 1)
        m["ab"] = ab[b0:b0 + BPC].reshape(T)
        m["pmat"] = P[b0:b0 + BPC].reshape(T, W)
        in_maps.append(m)
    return in_maps


def kernel(**inputs):
    nc = _get_program(L)
    in_maps = make_in_maps(inputs, L)
    res = run_bass_kernel_spmd(nc, in_maps, core_ids=list(range(N_CORES)))
    out = np.concatenate([res.results[i]["out"].reshape(BPC, S, C)
                          for i in range(N_CORES)], axis=0)
    return out.astype(np.float32)
